# revision 30
# baseline (speedup 1.0000x reference)
"""DeepSeek sparse attention on 8 Trainium2 NeuronCores (Bass/Tile).

Two SPMD launches:

  A (projections + lightning indexer, column/head-parallel): core c computes
     the 256-column slice (= its 2 attention heads) of q/k/v as fp16 (256,S)
     from f32r hidden^T resident in SBUF, PLUS the indexer-head-c projections
     qp_c/kp_c (256,S) using HOST-FUSED weights Wq@Wq_ind / Wk@Wk_ind (f32r,
     full precision — the top-k selection needs ~1e-5 relative accuracy), and
     rel_c[t] = sum_k relu(qp_c[t] . kp_c[k]) via PE + one relu-accumulate
     ACT pass per 128-token tile. Indexer passes run first so they overlap
     the hidden DMA window; scores interleave with the q/k/v passes.
  host: rel = sum_c w_c rel_c; top-1024 -> selected mask; hi/sel vectors.
  B (attention, head-parallel): core c feeds ITS OWN fp16 q/k/v slices from
     launch A straight back (no concat), computes causal/local/selected
     masked softmax attention for heads 2c,2c+1 and the partial output
     projection (S,H) in fp16; host sums the 8 fp16 partials in fp32.

All matmuls f32r or fp16 (1 PE cycle/row at N=512). fp16 everywhere in B
(calibrated: bf16 q/k/v + 16-bit partial store => 2.9e-3 rel err; fp16 is
strictly tighter; indexer stays f32r — one top-k swap costs ~1.5e-2).
"""

import math

import numpy as np

import concourse.bass as bass
from concourse import bass_isa
import concourse.mybir as mybir
from concourse import bacc
from concourse.tile import TileContext
from concourse.masks import make_identity
from concourse.bass_utils import run_bass_kernel_spmd

# Problem constants (hardcoded per contract)
HIDDEN = 2048
NUM_HEADS = 16
HEAD_DIM = 128
NUM_IND_HEADS = 8
IND_DIM = HIDDEN // NUM_IND_HEADS  # 256
MAX_SELECTED = 1024
LOCAL_WINDOW = 512
N_CORES = 8

F32 = mybir.dt.float32
F32R = mybir.dt.float32r
F16 = mybir.dt.float16
FP32 = np.float32

_TRACE = {"on": False, "exec_ns": []}


def build_a(S=2048, H=HIDDEN, CS=HIDDEN // N_CORES):
    """Per-core: q/k/v column slices (CS,S) fp16 + indexer rel_c (S) f32."""
    nc = bacc.Bacc("TRN2", target_bir_lowering=False, debug=False)
    HT, MC, NQ, QT = H // 128, CS // 128, S // 512, S // 128
    hidT = nc.dram_tensor("hidT", [H, S], F32R, kind="ExternalInput")
    wq = nc.dram_tensor("wq", [H, CS], F32R, kind="ExternalInput")
    wk = nc.dram_tensor("wk", [H, CS], F32R, kind="ExternalInput")
    wv = nc.dram_tensor("wv", [H, CS], F32R, kind="ExternalInput")
    wfq = nc.dram_tensor("wfq", [H, CS], F32R, kind="ExternalInput")
    wfk = nc.dram_tensor("wfk", [H, CS], F32R, kind="ExternalInput")
    qT = nc.dram_tensor("qT", [CS, S], F16, kind="ExternalOutput")
    kT = nc.dram_tensor("kT", [CS, S], F16, kind="ExternalOutput")
    vT = nc.dram_tensor("vT", [CS, S], F16, kind="ExternalOutput")
    rel = nc.dram_tensor("rel", [S], F32, kind="ExternalOutput")

    G = 8          # hidden chunks
    TG = HT // G   # strips per chunk

    with TileContext(nc) as tc:
        with (
            tc.tile_pool(name="hid", bufs=1) as hpool,
            tc.tile_pool(name="wt", bufs=2) as wpool,
            tc.tile_pool(name="proj", bufs=1) as ppool,
            tc.tile_pool(name="st", bufs=2) as stpool,
            tc.tile_pool(name="scr", bufs=1) as scrpool,
            tc.tile_pool(name="rm", bufs=1) as rmpool,
            tc.tile_pool(name="ps", bufs=1, space="PSUM") as pspool,
        ):
            # ---- input DMAs. Order matters: the first matmul needs the
            # first half of wfq plus hidden chunk 0, so those go first; wfk
            # comes after the hidden chunks (first needed ~50us in); wq/wk/wv
            # are issued later, at the program points where their weight-pool
            # slot is freed (avoids WAR stalls on the slot).
            def load_w(wdram, dt=F32R, halves=1):
                wr = wpool.tile([128, HT * CS], dt, tag="w", name="w")
                hh = HT // halves
                for i in range(halves):
                    nc.sync.dma_start(
                        out=wr[:, i * hh * CS:(i + 1) * hh * CS].rearrange(
                            "p (t c) -> p t c", t=hh
                        ),
                        in_=wdram[i * hh * 128:(i + 1) * hh * 128, :].rearrange(
                            "(t p) c -> p t c", p=128
                        ),
                    )
                return wr

            # hidden chunks: strips [1,1,2,2,2,2,2,2,2] so the first
            # matmul only waits on one 1MB strip; wfq loads in quarters.
            CHUNKS = [1, 1] + [2] * 7
            CUM = [0]
            for n in CHUNKS:
                CUM.append(CUM[-1] + n)
            strip2chunk = {}
            for g, n in enumerate(CHUNKS):
                for t in range(CUM[g], CUM[g + 1]):
                    strip2chunk[t] = (g, t - CUM[g])
            hidc = [hpool.tile([128, CHUNKS[g] * S], F32R, name=f"hidc{g}")
                    for g in range(len(CHUNKS))]

            def load_hid(g):
                n = CHUNKS[g]
                nc.sync.dma_start(
                    out=hidc[g].rearrange("p (t s) -> p t s", t=n),
                    in_=hidT[CUM[g] * 128:CUM[g + 1] * 128, :].rearrange(
                        "(t p) s -> p t s", p=128
                    ),
                )

            wfq_t = wpool.tile([128, HT * CS], F32R, tag="w", name="w")
            QQ = HT // 4
            nc.sync.dma_start(
                out=wfq_t[:, :QQ * CS].rearrange("p (t c) -> p t c", t=QQ),
                in_=wfq[:QQ * 128, :].rearrange("(t p) c -> p t c", p=128),
            )
            load_hid(0)
            nc.sync.dma_start(
                out=wfq_t[:, QQ * CS:2 * QQ * CS].rearrange(
                    "p (t c) -> p t c", t=QQ),
                in_=wfq[QQ * 128:2 * QQ * 128, :].rearrange(
                    "(t p) c -> p t c", p=128),
            )
            load_hid(1)
            nc.sync.dma_start(
                out=wfq_t[:, 2 * QQ * CS:].rearrange(
                    "p (t c) -> p t c", t=2 * QQ),
                in_=wfq[2 * QQ * 128:, :].rearrange(
                    "(t p) c -> p t c", p=128),
            )
            for g in range(2, len(CHUNKS)):
                load_hid(g)
            wfk_t = load_w(wfk)

            # resident f32r indexer projections qp^T/kp^T (2 x 128 x S each)
            qpt = [ppool.tile([128, S], F32R, name=f"qpt{m}") for m in range(MC)]
            kpt = [ppool.tile([128, S], F32R, name=f"kpt{m}") for m in range(MC)]

            # psum regions: 4 banks for projection passes ("pj"), and one
            # [128, S] region ("scr") that serves double duty: the kp passes
            # accumulate in its 512-slices, and the indexer-score tiles use
            # it whole.
            def pj_psums():
                return [
                    pspool.tile([128, 512], F32, tag=f"pj{i}", name=f"pj{i}")
                    for i in range(NQ)
                ]

            def scr_psum():
                return pspool.tile([128, S], F32, tag="scr", name="scr")

            relmat = rmpool.tile([128, QT], F32, name="relmat")
            scratch = scrpool.tile([128, S], F16, name="scratch")

            score_state = {"next": 0}

            def emit_score_qt():
                """Indexer scores for one 128-token tile: 8 matmuls into the
                scr psum region + one relu-accumulate -> relmat column."""
                qt = score_state["next"]
                if qt >= QT:
                    return False
                score_state["next"] += 1
                sps = scr_psum()
                for d in range(MC):
                    for kc in range(NQ):
                        nc.tensor.matmul(
                            sps[:, kc * 512:(kc + 1) * 512],
                            qpt[d][:, qt * 128:(qt + 1) * 128],
                            kpt[d][:, kc * 512:(kc + 1) * 512],
                            start=(d == 0), stop=(d == MC - 1),
                        )
                nc.scalar.activation(
                    scratch, sps, mybir.ActivationFunctionType.Relu,
                    accum_out=relmat[:, qt:qt + 1],
                )
                return True

            def sl(psums, qc):
                return (psums[qc] if isinstance(psums, list)
                        else psums[:, qc * 512:(qc + 1) * 512])

            def proj_passes(groups, score_slots=()):
                """Interleaved m-tile passes: each group = (wtile, mc, psums,
                finish). Strips advance together so every group progresses
                chunk-by-chunk behind the hidden DMA."""
                for t in range(HT):
                    for wtile, mc, psums, _ in groups:
                        lhsT = wtile[:, t * CS + mc * 128:
                                     t * CS + mc * 128 + 128]
                        g, tl = strip2chunk[t]
                        rhs = hidc[g]
                        for qc in range(NQ):
                            nc.tensor.matmul(
                                sl(psums, qc), lhsT,
                                rhs[:, tl * S + qc * 512:
                                    tl * S + qc * 512 + 512],
                                start=(t == 0), stop=(t == HT - 1),
                            )
                    if t in score_slots:
                        emit_score_qt()
                for _, _, _, finish in groups:
                    finish()

            def copy_to(dst, psums):
                def fin():
                    for qc in range(NQ):
                        eng = nc.vector if qc % 2 == 0 else nc.scalar
                        if eng is nc.vector:
                            nc.vector.tensor_copy(
                                dst[:, qc * 512:(qc + 1) * 512], sl(psums, qc)
                            )
                        else:
                            nc.scalar.copy(
                                dst[:, qc * 512:(qc + 1) * 512], sl(psums, qc)
                            )
                return fin

            # ---- indexer projection passes; the qp-m0/m1 pair overlaps the
            # hidden-DMA window (both read wfq, which arrives first).
            pj = pj_psums()
            sc = scr_psum()
            proj_passes([
                (wfq_t, 0, pj, copy_to(qpt[0], pj)),
                (wfq_t, 1, sc, copy_to(qpt[1], sc)),
            ])
            # wfq slot free now -> issue wq load
            wq_t = load_w(wq)
            pj = pj_psums()
            sc = scr_psum()
            proj_passes([
                (wfk_t, 0, pj, copy_to(kpt[0], pj)),
                (wfk_t, 1, sc, copy_to(kpt[1], sc)),
            ])
            wk_t = load_w(wk)

            # ---- q/k/v passes (fp16 weights) with indexer scores interleaved
            def store_pass(wtile, mc, odram):
                psums = pj_psums()
                stage = stpool.tile([128, S], F16, tag="st", name="st")

                def fin():
                    for qc in range(NQ):
                        if qc % 2 == 0:
                            nc.vector.tensor_copy(
                                stage[:, qc * 512:(qc + 1) * 512], psums[qc]
                            )
                        else:
                            nc.scalar.copy(
                                stage[:, qc * 512:(qc + 1) * 512], psums[qc]
                            )
                    nc.sync.dma_start(
                        out=odram[mc * 128:(mc + 1) * 128, :], in_=stage
                    )
                proj_passes([(wtile, mc, psums, fin)],
                            score_slots=(1, 6, 11))

            store_pass(wq_t, 0, qT)
            store_pass(wq_t, 1, qT)
            wv_t = load_w(wv)
            store_pass(wk_t, 0, kT)
            store_pass(wk_t, 1, kT)
            store_pass(wv_t, 0, vT)
            store_pass(wv_t, 1, vT)
            while emit_score_qt():
                pass

            nc.sync.dma_start(
                out=rel.rearrange("(t p) -> p t", p=128), in_=relmat
            )
    nc.compile()
    return nc


def build_b(S=2048, H=HIDDEN, NHC=NUM_HEADS // N_CORES, HD=HEAD_DIM,
            window=LOCAL_WINDOW):
    """Per-core attention for 2 heads + fp16 partial output projection."""
    nc = bacc.Bacc("TRN2", target_bir_lowering=False, debug=False)
    KC, NQ, QT, OCC = S // 128, S // 512, S // 128, H // 512
    qTh = nc.dram_tensor("qTh", [NHC * HD, S], F16, kind="ExternalInput")
    kTh = nc.dram_tensor("kTh", [NHC * HD, S], F16, kind="ExternalInput")
    vTh = nc.dram_tensor("vTh", [NHC * HD, S], F16, kind="ExternalInput")
    woh = nc.dram_tensor("woh", [NHC * HD, H], F16, kind="ExternalInput")
    hivec = nc.dram_tensor("hivec", [S], F16, kind="ExternalInput")
    selv = nc.dram_tensor("selv", [S], F16, kind="ExternalInput")
    part = nc.dram_tensor("part", [S, H], F16, kind="ExternalOutput")

    scale = 1.0 / math.sqrt(HD)
    AF = mybir.ActivationFunctionType
    OP = mybir.AluOpType

    with TileContext(nc) as tc:
        with (
            tc.tile_pool(name="const", bufs=1) as cpool,
            tc.tile_pool(name="qk", bufs=1) as qkpool,
            tc.tile_pool(name="vt", bufs=2) as vtpool,
            tc.tile_pool(name="vh", bufs=1) as vhpool,
            tc.tile_pool(name="et", bufs=4) as etpool,
            tc.tile_pool(name="aon", bufs=1) as aopool,
            tc.tile_pool(name="dr", bufs=2) as drpool,
            tc.tile_pool(name="ost", bufs=2) as ostpool,
            tc.tile_pool(name="ps", bufs=1, space="PSUM") as pspool,
        ):
            # v first (transposes are at the head of the PE queue), then q/k
            # of head 0 so scoring starts right behind the transposes.
            vts0 = vtpool.tile([128, S], F16, tag="vts", name="vts")
            nc.sync.dma_start(out=vts0, in_=vTh[0:HD, :])
            qsb, ksb = [], []
            for h in range(NHC):
                k = qkpool.tile([128, S], F16, name=f"ksb{h}")
                nc.sync.dma_start(out=k, in_=kTh[h * HD:(h + 1) * HD, :])
                ksb.append(k)
                q = qkpool.tile([128, S], F16, name=f"qsb{h}")
                nc.sync.dma_start(out=q, in_=qTh[h * HD:(h + 1) * HD, :])
                qsb.append(q)

            hvec = cpool.tile([128, KC], F16, name="hvec")
            nc.sync.dma_start(out=hvec, in_=hivec.rearrange("(t p) -> p t", p=128))
            svec = cpool.tile([128, KC], F16, name="svec")
            nc.sync.dma_start(out=svec, in_=selv.rearrange("(t p) -> p t", p=128))

            wsb = []
            for h in range(NHC):
                w = qkpool.tile([128, H], F16, name=f"wsb{h}")
                nc.sync.dma_start(out=w, in_=woh[h * HD:(h + 1) * HD, :])
                wsb.append(w)

            svec32 = cpool.tile([128, KC], F32, name="svec32")
            nc.vector.tensor_copy(svec32, svec)
            ones = cpool.tile([128, 1], F16, name="ones")
            nc.vector.memset(ones, 1.0)
            ident = cpool.tile([128, 128], F16, name="ident")
            make_identity(nc, ident)
            iota = cpool.tile([128, S], F16, name="iota")
            nc.gpsimd.iota(
                iota, pattern=[[1, S]], base=0, channel_multiplier=0,
                allow_small_or_imprecise_dtypes=True,
            )

            aon = [aopool.tile([128, S], F16, name=f"aon{h}") for h in range(NHC)]
            vhf = [vhpool.tile([128, S], F16, name=f"vhf{h}") for h in range(NHC)]
            vsl = [vhpool.tile([128, S], F16, name=f"vsl{h}") for h in range(NHC)]

            def normalize(h, qc, avp, den):
                q0 = qc * 512
                dq = drpool.tile([1, 512], F32, tag="dq", name="dq")
                nc.scalar.copy(dq, den[0:1, :])
                rq = drpool.tile([1, 512], F32, tag="rq", name="rq")
                rs = drpool.tile([1, 512], F32, tag="rs", name="rs")
                nc.vector.reciprocal_approx_accurate(rq, dq, rs)
                rbs = drpool.tile([128, 512], F32, tag="rbs", name="rbs")
                nc.gpsimd.partition_broadcast(rbs, rq)
                nc.vector.scalar_tensor_tensor(
                    aon[h][:, q0:q0 + 512], rbs, 1.0, avp,
                    op0=OP.mult, op1=OP.mult,
                )

            def outproj(qc):
                """Output projection for the 4 query tiles of chunk qc,
                accumulating both heads; fp16 stage -> one DMA per tile."""
                for qt in range(qc * NQ, qc * NQ + NQ):
                    ostage = ostpool.tile([128, H], F16, tag="ost", name="ost")
                    for oc in range(OCC):
                        wop = pspool.tile([128, 512], F32, tag="wo", bufs=2,
                                          name="wo")
                        for h in range(NHC):
                            nc.tensor.matmul(
                                wop, aon[h][:, qt * 128:(qt + 1) * 128],
                                wsb[h][:, oc * 512:(oc + 1) * 512],
                                start=(h == 0), stop=(h == NHC - 1),
                            )
                        if oc % 2 == 0:
                            nc.vector.tensor_copy(
                                ostage[:, oc * 512:(oc + 1) * 512], wop
                            )
                        else:
                            nc.scalar.copy(
                                ostage[:, oc * 512:(oc + 1) * 512], wop
                            )
                    nc.sync.dma_start(
                        out=part[qt * 128:(qt + 1) * 128, :], in_=ostage
                    )

            for h in range(NHC):
                if h == 0:
                    vts = vts0
                else:
                    vts = vtpool.tile([128, S], F16, tag="vts", name="vts")
                    nc.sync.dma_start(out=vts, in_=vTh[h * HD:(h + 1) * HD, :])

                def transpose_batch(kcs):
                    # v tiles -> (k, hd) layout; vsl = v * selected. The tp
                    # psum shares the "wo" tag (outproj is temporally
                    # disjoint); copies go to gpsimd, which is mostly idle.
                    for kc in kcs:
                        tp = pspool.tile([128, 128], F16, tag="wo", bufs=2,
                                         name="tp")
                        nc.tensor.transpose(
                            tp, vts[:, kc * 128:(kc + 1) * 128], ident
                        )
                        if kc % 2 == 0:
                            nc.vector.tensor_copy(
                                vhf[h][:, kc * 128:(kc + 1) * 128], tp
                            )
                        else:
                            nc.scalar.copy(
                                vhf[h][:, kc * 128:(kc + 1) * 128], tp
                            )
                        nc.vector.tensor_scalar_mul(
                            vsl[h][:, kc * 128:(kc + 1) * 128],
                            vhf[h][:, kc * 128:(kc + 1) * 128],
                            svec32[:, kc:kc + 1],
                        )

                for qc in range(NQ):
                    q0 = qc * 512
                    kcm = (q0 + 511) // 128  # last causal k-tile
                    avp = pspool.tile([128, 512], F32, tag="av", bufs=2,
                                      name="avp")
                    den = pspool.tile([128, 512], F32, tag="den", bufs=1,
                                      name="den")

                    def score_tile(kc):
                        k0 = kc * 128
                        far = q0 > k0 + 127 + window
                        sps = pspool.tile([128, 512], F32, tag="sc", bufs=3,
                                          name="sps")
                        nc.tensor.matmul(
                            sps, ksb[h][:, k0:k0 + 128],
                            qsb[h][:, q0:q0 + 512],
                            start=True, stop=True,
                        )
                        et = etpool.tile([128, 512], F16, tag="et", name="et")
                        nc.scalar.activation(et, sps, AF.Exp, scale=scale)
                        if q0 < k0 + 128:
                            nc.gpsimd.affine_select(
                                out=et, in_=et, compare_op=OP.is_ge, fill=0.0,
                                base=q0 - k0, channel_multiplier=-1,
                                pattern=[[1, 512]],
                            )
                        elif not far and q0 + 511 > k0 + window:
                            nc.vector.scalar_tensor_tensor(
                                et, iota[:, q0:q0 + 512], hvec[:, kc:kc + 1],
                                et, op0=OP.is_le, op1=OP.mult,
                            )
                        return et, far

                    def av_den(kc, et, far):
                        k0 = kc * 128
                        nc.tensor.matmul(
                            avp, (vsl if far else vhf)[h][:, k0:k0 + 128], et,
                            start=(kc == 0), stop=(kc == kcm),
                        )
                        nc.tensor.matmul(
                            den[0:1, :],
                            svec[:, kc:kc + 1] if far else ones, et,
                            start=(kc == 0), stop=(kc == kcm),
                            tile_position=(0, 0),
                        )

                    if qc == 0:
                        # scores first: they only need q/k, so the PE can
                        # work while v arrives and the transposes run.
                        ets = [score_tile(kc) for kc in range(kcm + 1)]
                        transpose_batch(range(qc * NQ, qc * NQ + NQ))
                        for kc, (et, far) in enumerate(ets):
                            av_den(kc, et, far)
                    else:
                        transpose_batch(range(qc * NQ, qc * NQ + NQ))
                        for kc in range(kcm + 1):
                            et, far = score_tile(kc)
                            av_den(kc, et, far)
                    normalize(h, qc, avp, den)
                    if h == NHC - 1:
                        outproj(qc)
    nc.compile()
    return nc


_CACHE = {}


def _get(name, builder, *args):
    key = (name,) + args
    if key not in _CACHE:
        _CACHE[key] = builder(*args)
    return _CACHE[key]


def _run(nc, in_maps):
    res = run_bass_kernel_spmd(
        nc, in_maps, core_ids=list(range(N_CORES)), trace=_TRACE["on"]
    )
    if _TRACE["on"] and res.exec_time_ns is not None:
        _TRACE["exec_ns"].append(res.exec_time_ns)
    return res.results


def kernel(hidden_states, Wq, Wk, Wv, Wo, Wq_ind, Wk_ind, head_weights,
           temperature_param):
    hidden_states = np.asarray(hidden_states, dtype=FP32)
    Wq, Wk, Wv, Wo = (np.asarray(a, dtype=FP32) for a in (Wq, Wk, Wv, Wo))
    Wq_ind = np.asarray(Wq_ind, dtype=FP32)
    Wk_ind = np.asarray(Wk_ind, dtype=FP32)
    head_weights = np.asarray(head_weights, dtype=FP32)

    B, S, H = hidden_states.shape
    assert B == 1 and H == HIDDEN
    CS = H // N_CORES
    hidT = np.ascontiguousarray(hidden_states[0].T)
    # host-fused indexer weights (fp64 for exactness)
    Wfq = (Wq.astype(np.float64) @ Wq_ind.astype(np.float64)).astype(FP32)
    Wfk = (Wk.astype(np.float64) @ Wk_ind.astype(np.float64)).astype(FP32)

    # ---- launch A ----
    nca = _get("a", build_a, S, H, CS)
    ina = [
        {
            "hidT": hidT,
            "wq": np.ascontiguousarray(Wq[:, c * CS:(c + 1) * CS]),
            "wk": np.ascontiguousarray(Wk[:, c * CS:(c + 1) * CS]),
            "wv": np.ascontiguousarray(Wv[:, c * CS:(c + 1) * CS]),
            "wfq": np.ascontiguousarray(Wfq[:, c * CS:(c + 1) * CS]),
            "wfk": np.ascontiguousarray(Wfk[:, c * CS:(c + 1) * CS]),
        }
        for c in range(N_CORES)
    ]
    ra = _run(nca, ina)

    rel = np.zeros(S, dtype=np.float64)
    for c in range(N_CORES):
        rel += float(head_weights[c]) * ra[c]["rel"].astype(np.float64)
    # exp(-temp) scaling is monotone; irrelevant for top-k selection.

    k_sel = min(MAX_SELECTED, S)
    top_idx = np.argpartition(-rel, k_sel - 1)[:k_sel]
    selected = np.zeros(S, dtype=bool)
    selected[top_idx] = True

    # ---- launch B ----
    BIG = float(2 * S + 1024)
    hi = np.where(selected, BIG, np.arange(S, dtype=np.float64) + LOCAL_WINDOW)
    inb = [
        {
            "qTh": ra[c]["qT"],
            "kTh": ra[c]["kT"],
            "vTh": ra[c]["vT"],
            "woh": Wo[c * CS:(c + 1) * CS].astype(np.float16),
            "hivec": hi.astype(np.float16),
            "selv": selected.astype(np.float16),
        }
        for c in range(N_CORES)
    ]
    ncb = _get("b", build_b, S, H, NUM_HEADS // N_CORES, HEAD_DIM, LOCAL_WINDOW)
    rb = _run(ncb, inb)
    out = np.zeros((S, H), dtype=np.float32)
    for c in range(N_CORES):
        out += rb[c]["part"].astype(np.float32)
    return out.reshape(B, S, H)


# revision 31
# speedup vs baseline: 1.1806x; 1.1806x over previous
"""DeepSeek sparse attention on 8 Trainium2 NeuronCores (Bass/Tile).

Two SPMD launches:

  A (projections + lightning indexer, column/head-parallel): core c computes
     the 256-column slice (= its 2 attention heads) of q/k/v as fp16 (256,S)
     from f32r hidden^T resident in SBUF, PLUS the indexer-head-c projections
     qp_c/kp_c (256,S) using HOST-FUSED weights Wq@Wq_ind / Wk@Wk_ind (f32r,
     full precision — the top-k selection needs ~1e-5 relative accuracy), and
     rel_c[t] = sum_k relu(qp_c[t] . kp_c[k]) via PE + one relu-accumulate
     ACT pass per 128-token tile. Indexer passes run first so they overlap
     the hidden DMA window; scores interleave with the q/k/v passes.
  host: rel = sum_c w_c rel_c; top-1024 -> selected mask; hi/sel vectors.
  B (attention, head-parallel): core c feeds ITS OWN fp16 q/k/v slices from
     launch A straight back (no concat), computes causal/local/selected
     masked softmax attention for heads 2c,2c+1 and the partial output
     projection (S,H) in fp16; host sums the 8 fp16 partials in fp32.

All matmuls f32r or fp16 (1 PE cycle/row at N=512). fp16 everywhere in B
(calibrated: bf16 q/k/v + 16-bit partial store => 2.9e-3 rel err; fp16 is
strictly tighter; indexer stays f32r — one top-k swap costs ~1.5e-2).
"""

import math

import numpy as np

import concourse.bass as bass
from concourse import bass_isa
import concourse.mybir as mybir
from concourse import bacc
from concourse.tile import TileContext
from concourse.masks import make_identity
from concourse.bass_utils import run_bass_kernel_spmd

# Problem constants (hardcoded per contract)
HIDDEN = 2048
NUM_HEADS = 16
HEAD_DIM = 128
NUM_IND_HEADS = 8
IND_DIM = HIDDEN // NUM_IND_HEADS  # 256
MAX_SELECTED = 1024
LOCAL_WINDOW = 512
N_CORES = 8

F32 = mybir.dt.float32
F32R = mybir.dt.float32r
F16 = mybir.dt.float16
FP32 = np.float32

_TRACE = {"on": False, "exec_ns": []}


def build_a(S=2048, H=HIDDEN, CS=HIDDEN // N_CORES):
    """Per-core: q/k/v column slices (CS,S) fp16 + indexer rel_c (S) f32."""
    nc = bacc.Bacc("TRN2", target_bir_lowering=False, debug=False)
    HT, MC, NQ, QT = H // 128, CS // 128, S // 512, S // 128
    hidT = nc.dram_tensor("hidT", [H, S], F32R, kind="ExternalInput")
    wq = nc.dram_tensor("wq", [H, CS], F32R, kind="ExternalInput")
    wk = nc.dram_tensor("wk", [H, CS], F32R, kind="ExternalInput")
    wv = nc.dram_tensor("wv", [H, CS], F32R, kind="ExternalInput")
    wfq = nc.dram_tensor("wfq", [H, CS], F32R, kind="ExternalInput")
    wfk = nc.dram_tensor("wfk", [H, CS], F32R, kind="ExternalInput")
    qT = nc.dram_tensor("qT", [CS, S], F16, kind="ExternalOutput")
    kT = nc.dram_tensor("kT", [CS, S], F16, kind="ExternalOutput")
    vT = nc.dram_tensor("vT", [CS, S], F16, kind="ExternalOutput")
    rel = nc.dram_tensor("rel", [S], F32, kind="ExternalOutput")

    G = 8          # hidden chunks
    TG = HT // G   # strips per chunk

    with TileContext(nc) as tc:
        with (
            tc.tile_pool(name="hid", bufs=1) as hpool,
            tc.tile_pool(name="wt", bufs=2) as wpool,
            tc.tile_pool(name="proj", bufs=1) as ppool,
            tc.tile_pool(name="st", bufs=2) as stpool,
            tc.tile_pool(name="scr", bufs=1) as scrpool,
            tc.tile_pool(name="rm", bufs=1) as rmpool,
            tc.tile_pool(name="ps", bufs=1, space="PSUM") as pspool,
        ):
            # ---- input DMAs. Order matters: the first matmul needs the
            # first half of wfq plus hidden chunk 0, so those go first; wfk
            # comes after the hidden chunks (first needed ~50us in); wq/wk/wv
            # are issued later, at the program points where their weight-pool
            # slot is freed (avoids WAR stalls on the slot).
            def load_w(wdram, dt=F32R, halves=1):
                wr = wpool.tile([128, HT * CS], dt, tag="w", name="w")
                hh = HT // halves
                for i in range(halves):
                    nc.sync.dma_start(
                        out=wr[:, i * hh * CS:(i + 1) * hh * CS].rearrange(
                            "p (t c) -> p t c", t=hh
                        ),
                        in_=wdram[i * hh * 128:(i + 1) * hh * 128, :].rearrange(
                            "(t p) c -> p t c", p=128
                        ),
                    )
                return wr

            # hidden chunks: strips [1,1,2,2,2,2,2,2,2] so the first
            # matmul only waits on one 1MB strip; wfq loads in quarters.
            CHUNKS = [1, 1] + [2] * 7
            CUM = [0]
            for n in CHUNKS:
                CUM.append(CUM[-1] + n)
            strip2chunk = {}
            for g, n in enumerate(CHUNKS):
                for t in range(CUM[g], CUM[g + 1]):
                    strip2chunk[t] = (g, t - CUM[g])
            hidc = [hpool.tile([128, CHUNKS[g] * S], F32R, name=f"hidc{g}")
                    for g in range(len(CHUNKS))]

            def load_hid(g):
                n = CHUNKS[g]
                nc.sync.dma_start(
                    out=hidc[g].rearrange("p (t s) -> p t s", t=n),
                    in_=hidT[CUM[g] * 128:CUM[g + 1] * 128, :].rearrange(
                        "(t p) s -> p t s", p=128
                    ),
                )

            wfq_t = wpool.tile([128, HT * CS], F32R, tag="w", name="w")
            QQ = HT // 4
            nc.sync.dma_start(
                out=wfq_t[:, :QQ * CS].rearrange("p (t c) -> p t c", t=QQ),
                in_=wfq[:QQ * 128, :].rearrange("(t p) c -> p t c", p=128),
            )
            load_hid(0)
            nc.sync.dma_start(
                out=wfq_t[:, QQ * CS:2 * QQ * CS].rearrange(
                    "p (t c) -> p t c", t=QQ),
                in_=wfq[QQ * 128:2 * QQ * 128, :].rearrange(
                    "(t p) c -> p t c", p=128),
            )
            load_hid(1)
            nc.sync.dma_start(
                out=wfq_t[:, 2 * QQ * CS:].rearrange(
                    "p (t c) -> p t c", t=2 * QQ),
                in_=wfq[2 * QQ * 128:, :].rearrange(
                    "(t p) c -> p t c", p=128),
            )
            for g in range(2, len(CHUNKS)):
                load_hid(g)
            wfk_t = load_w(wfk)

            # resident f32r indexer projections qp^T/kp^T (2 x 128 x S each)
            qpt = [ppool.tile([128, S], F32R, name=f"qpt{m}") for m in range(MC)]
            kpt = [ppool.tile([128, S], F32R, name=f"kpt{m}") for m in range(MC)]

            # psum regions: 4 banks for projection passes ("pj"), and one
            # [128, S] region ("scr") that serves double duty: the kp passes
            # accumulate in its 512-slices, and the indexer-score tiles use
            # it whole.
            def pj_psums():
                return [
                    pspool.tile([128, 512], F32, tag=f"pj{i}", name=f"pj{i}")
                    for i in range(NQ)
                ]

            def scr_psum():
                return pspool.tile([128, S], F32, tag="scr", name="scr")

            relmat = rmpool.tile([128, QT], F32, name="relmat")
            scratch = scrpool.tile([128, S], F16, name="scratch")

            score_state = {"next": 0}

            def emit_score_qt():
                """Indexer scores for one 128-token tile: 8 matmuls into the
                scr psum region + one relu-accumulate -> relmat column."""
                qt = score_state["next"]
                if qt >= QT:
                    return False
                score_state["next"] += 1
                sps = scr_psum()
                for d in range(MC):
                    for kc in range(NQ):
                        nc.tensor.matmul(
                            sps[:, kc * 512:(kc + 1) * 512],
                            qpt[d][:, qt * 128:(qt + 1) * 128],
                            kpt[d][:, kc * 512:(kc + 1) * 512],
                            start=(d == 0), stop=(d == MC - 1),
                        )
                nc.scalar.activation(
                    scratch, sps, mybir.ActivationFunctionType.Relu,
                    accum_out=relmat[:, qt:qt + 1],
                )
                return True

            def sl(psums, qc):
                return (psums[qc] if isinstance(psums, list)
                        else psums[:, qc * 512:(qc + 1) * 512])

            def proj_passes(groups, score_slots=()):
                """Interleaved m-tile passes: each group = (wtile, mc, psums,
                finish). Strips advance together so every group progresses
                chunk-by-chunk behind the hidden DMA."""
                for t in range(HT):
                    for wtile, mc, psums, _ in groups:
                        lhsT = wtile[:, t * CS + mc * 128:
                                     t * CS + mc * 128 + 128]
                        g, tl = strip2chunk[t]
                        rhs = hidc[g]
                        for qc in range(NQ):
                            nc.tensor.matmul(
                                sl(psums, qc), lhsT,
                                rhs[:, tl * S + qc * 512:
                                    tl * S + qc * 512 + 512],
                                start=(t == 0), stop=(t == HT - 1),
                            )
                    if t in score_slots:
                        emit_score_qt()
                for _, _, _, finish in groups:
                    finish()

            def copy_to(dst, psums):
                def fin():
                    for qc in range(NQ):
                        eng = nc.vector if qc % 2 == 0 else nc.scalar
                        if eng is nc.vector:
                            nc.vector.tensor_copy(
                                dst[:, qc * 512:(qc + 1) * 512], sl(psums, qc)
                            )
                        else:
                            nc.scalar.copy(
                                dst[:, qc * 512:(qc + 1) * 512], sl(psums, qc)
                            )
                return fin

            # ---- indexer projection passes; the qp-m0/m1 pair overlaps the
            # hidden-DMA window (both read wfq, which arrives first).
            pj = pj_psums()
            sc = scr_psum()
            proj_passes([
                (wfq_t, 0, pj, copy_to(qpt[0], pj)),
                (wfq_t, 1, sc, copy_to(qpt[1], sc)),
            ])
            # wfq slot free now -> issue wq load
            wq_t = load_w(wq)
            pj = pj_psums()
            sc = scr_psum()
            proj_passes([
                (wfk_t, 0, pj, copy_to(kpt[0], pj)),
                (wfk_t, 1, sc, copy_to(kpt[1], sc)),
            ])
            wk_t = load_w(wk)

            # ---- q/k/v passes (fp16 weights) with indexer scores interleaved
            def store_pass(wtile, mc, odram):
                psums = pj_psums()
                stage = stpool.tile([128, S], F16, tag="st", name="st")

                def fin():
                    for qc in range(NQ):
                        if qc % 2 == 0:
                            nc.vector.tensor_copy(
                                stage[:, qc * 512:(qc + 1) * 512], psums[qc]
                            )
                        else:
                            nc.scalar.copy(
                                stage[:, qc * 512:(qc + 1) * 512], psums[qc]
                            )
                    nc.sync.dma_start(
                        out=odram[mc * 128:(mc + 1) * 128, :], in_=stage
                    )
                proj_passes([(wtile, mc, psums, fin)],
                            score_slots=(1, 6, 11))

            store_pass(wq_t, 0, qT)
            store_pass(wq_t, 1, qT)
            wv_t = load_w(wv)
            store_pass(wk_t, 0, kT)
            store_pass(wk_t, 1, kT)
            store_pass(wv_t, 0, vT)
            store_pass(wv_t, 1, vT)
            while emit_score_qt():
                pass

            nc.sync.dma_start(
                out=rel.rearrange("(t p) -> p t", p=128), in_=relmat
            )
    nc.compile()
    return nc


def build_b(S=2048, H=HIDDEN, NHC=NUM_HEADS // N_CORES, HD=HEAD_DIM,
            window=LOCAL_WINDOW):
    """Per-core attention for 2 heads + fp16 partial output projection."""
    nc = bacc.Bacc("TRN2", target_bir_lowering=False, debug=False)
    KC, NQ, QT, OCC = S // 128, S // 512, S // 128, H // 512
    qTh = nc.dram_tensor("qTh", [NHC * HD, S], F16, kind="ExternalInput")
    kTh = nc.dram_tensor("kTh", [NHC * HD, S], F16, kind="ExternalInput")
    vTh = nc.dram_tensor("vTh", [NHC * HD, S], F16, kind="ExternalInput")
    woh = nc.dram_tensor("woh", [NHC * HD, H], F16, kind="ExternalInput")
    hivec = nc.dram_tensor("hivec", [S], F16, kind="ExternalInput")
    selv = nc.dram_tensor("selv", [S], F16, kind="ExternalInput")
    part = nc.dram_tensor("part", [S, H], F16, kind="ExternalOutput")

    scale = 1.0 / math.sqrt(HD)
    AF = mybir.ActivationFunctionType
    OP = mybir.AluOpType

    with TileContext(nc) as tc:
        with (
            tc.tile_pool(name="const", bufs=1) as cpool,
            tc.tile_pool(name="qk", bufs=1) as qkpool,
            tc.tile_pool(name="vt", bufs=2) as vtpool,
            tc.tile_pool(name="vh", bufs=1) as vhpool,
            tc.tile_pool(name="et", bufs=4) as etpool,
            tc.tile_pool(name="aon", bufs=1) as aopool,
            tc.tile_pool(name="dr", bufs=2) as drpool,
            tc.tile_pool(name="ost", bufs=2) as ostpool,
            tc.tile_pool(name="ps", bufs=1, space="PSUM") as pspool,
        ):
            # v first (transposes are at the head of the PE queue), then q/k
            # of head 0 so scoring starts right behind the transposes.
            vts0 = vtpool.tile([128, S], F16, tag="vts", name="vts")
            nc.sync.dma_start(out=vts0, in_=vTh[0:HD, :])
            qsb, ksb = [], []
            for h in range(NHC):
                k = qkpool.tile([128, S], F16, name=f"ksb{h}")
                nc.sync.dma_start(out=k, in_=kTh[h * HD:(h + 1) * HD, :])
                ksb.append(k)
                q = qkpool.tile([128, S], F16, name=f"qsb{h}")
                nc.sync.dma_start(out=q, in_=qTh[h * HD:(h + 1) * HD, :])
                qsb.append(q)

            hvec = cpool.tile([128, KC], F16, name="hvec")
            nc.sync.dma_start(out=hvec, in_=hivec.rearrange("(t p) -> p t", p=128))
            svec = cpool.tile([128, KC], F16, name="svec")
            nc.sync.dma_start(out=svec, in_=selv.rearrange("(t p) -> p t", p=128))

            wsb = []
            for h in range(NHC):
                w = qkpool.tile([128, H], F16, name=f"wsb{h}")
                nc.sync.dma_start(out=w, in_=woh[h * HD:(h + 1) * HD, :])
                wsb.append(w)

            svec32 = cpool.tile([128, KC], F32, name="svec32")
            nc.vector.tensor_copy(svec32, svec)
            ones = cpool.tile([128, 1], F16, name="ones")
            nc.vector.memset(ones, 1.0)
            ident = cpool.tile([128, 128], F16, name="ident")
            make_identity(nc, ident)
            iota = cpool.tile([128, S], F16, name="iota")
            nc.gpsimd.iota(
                iota, pattern=[[1, S]], base=0, channel_multiplier=0,
                allow_small_or_imprecise_dtypes=True,
            )
            kvec = cpool.tile([128, KC], F16, name="kvec")
            nc.gpsimd.iota(
                kvec, pattern=[[128, KC]], base=0, channel_multiplier=1,
                allow_small_or_imprecise_dtypes=True,
            )

            aon = [aopool.tile([128, S], F16, name=f"aon{h}") for h in range(NHC)]
            vhf = [vhpool.tile([128, S], F16, name=f"vhf{h}") for h in range(NHC)]
            vsl = [vhpool.tile([128, S], F16, name=f"vsl{h}") for h in range(NHC)]

            def normalize(h, qc, avp, den):
                q0 = qc * 512
                dq = drpool.tile([1, 512], F32, tag="dq", name="dq")
                nc.scalar.copy(dq, den[0:1, :])
                rq = drpool.tile([1, 512], F32, tag="rq", name="rq")
                rs = drpool.tile([1, 512], F32, tag="rs", name="rs")
                nc.vector.reciprocal_approx_accurate(rq, dq, rs)
                rbs = drpool.tile([128, 512], F32, tag="rbs", name="rbs")
                nc.gpsimd.partition_broadcast(rbs, rq)
                nc.vector.scalar_tensor_tensor(
                    aon[h][:, q0:q0 + 512], rbs, 1.0, avp,
                    op0=OP.mult, op1=OP.mult,
                )

            def outproj(qc):
                """Output projection for the 4 query tiles of chunk qc,
                accumulating both heads; fp16 stage -> one DMA per tile."""
                for qt in range(qc * NQ, qc * NQ + NQ):
                    ostage = ostpool.tile([128, H], F16, tag="ost", name="ost")
                    for oc in range(OCC):
                        wop = pspool.tile([128, 512], F32, tag="wo", bufs=2,
                                          name="wo")
                        for h in range(NHC):
                            nc.tensor.matmul(
                                wop, aon[h][:, qt * 128:(qt + 1) * 128],
                                wsb[h][:, oc * 512:(oc + 1) * 512],
                                start=(h == 0), stop=(h == NHC - 1),
                            )
                        if oc % 2 == 0:
                            nc.vector.tensor_copy(
                                ostage[:, oc * 512:(oc + 1) * 512], wop
                            )
                        else:
                            nc.scalar.copy(
                                ostage[:, oc * 512:(oc + 1) * 512], wop
                            )
                    nc.sync.dma_start(
                        out=part[qt * 128:(qt + 1) * 128, :], in_=ostage
                    )

            for h in range(NHC):
                if h == 0:
                    vts = vts0
                else:
                    vts = vtpool.tile([128, S], F16, tag="vts", name="vts")
                    nc.sync.dma_start(out=vts, in_=vTh[h * HD:(h + 1) * HD, :])

                def transpose_batch(kcs):
                    # v tiles -> (k, hd) layout; vsl = v * selected. The tp
                    # psum shares the "wo" tag (outproj is temporally
                    # disjoint); copies go to gpsimd, which is mostly idle.
                    for kc in kcs:
                        tp = pspool.tile([128, 128], F16, tag="wo", bufs=2,
                                         name="tp")
                        nc.tensor.transpose(
                            tp, vts[:, kc * 128:(kc + 1) * 128], ident
                        )
                        if kc % 2 == 0:
                            nc.vector.tensor_copy(
                                vhf[h][:, kc * 128:(kc + 1) * 128], tp
                            )
                        else:
                            nc.scalar.copy(
                                vhf[h][:, kc * 128:(kc + 1) * 128], tp
                            )
                        nc.vector.tensor_scalar_mul(
                            vsl[h][:, kc * 128:(kc + 1) * 128],
                            vhf[h][:, kc * 128:(kc + 1) * 128],
                            svec32[:, kc:kc + 1],
                        )

                for qc in range(NQ):
                    q0 = qc * 512
                    kcm = (q0 + 511) // 128  # last causal k-tile
                    avp = pspool.tile([128, 512], F32, tag="av", bufs=2,
                                      name="avp")
                    den = pspool.tile([128, 512], F32, tag="den", bufs=1,
                                      name="den")

                    def score_tile(kc):
                        k0 = kc * 128
                        far = q0 > k0 + 127 + window
                        sps = pspool.tile([128, 512], F32, tag="sc", bufs=3,
                                          name="sps")
                        nc.tensor.matmul(
                            sps, ksb[h][:, k0:k0 + 128],
                            qsb[h][:, q0:q0 + 512],
                            start=True, stop=True,
                        )
                        et = etpool.tile([128, 512], F16, tag="et", name="et")
                        nc.scalar.activation(et, sps, AF.Exp, scale=scale)
                        if q0 < k0 + 128:
                            nc.vector.scalar_tensor_tensor(
                                et, iota[:, q0:q0 + 512], kvec[:, kc:kc + 1],
                                et, op0=OP.is_ge, op1=OP.mult,
                            )
                        elif not far and q0 + 511 > k0 + window:
                            nc.vector.scalar_tensor_tensor(
                                et, iota[:, q0:q0 + 512], hvec[:, kc:kc + 1],
                                et, op0=OP.is_le, op1=OP.mult,
                            )
                        return et, far

                    def av_den(kc, et, far):
                        k0 = kc * 128
                        nc.tensor.matmul(
                            avp, (vsl if far else vhf)[h][:, k0:k0 + 128], et,
                            start=(kc == 0), stop=(kc == kcm),
                        )
                        nc.tensor.matmul(
                            den[0:1, :],
                            svec[:, kc:kc + 1] if far else ones, et,
                            start=(kc == 0), stop=(kc == kcm),
                            tile_position=(0, 0),
                        )

                    if qc == 0:
                        # scores first: they only need q/k, so the PE can
                        # work while v arrives and the transposes run.
                        ets = [score_tile(kc) for kc in range(kcm + 1)]
                        transpose_batch(range(qc * NQ, qc * NQ + NQ))
                        for kc, (et, far) in enumerate(ets):
                            av_den(kc, et, far)
                    else:
                        transpose_batch(range(qc * NQ, qc * NQ + NQ))
                        for kc in range(kcm + 1):
                            et, far = score_tile(kc)
                            av_den(kc, et, far)
                    normalize(h, qc, avp, den)
                    if h == NHC - 1:
                        outproj(qc)
    nc.compile()
    return nc


_CACHE = {}


def _get(name, builder, *args):
    key = (name,) + args
    if key not in _CACHE:
        _CACHE[key] = builder(*args)
    return _CACHE[key]


def _run(nc, in_maps):
    res = run_bass_kernel_spmd(
        nc, in_maps, core_ids=list(range(N_CORES)), trace=_TRACE["on"]
    )
    if _TRACE["on"] and res.exec_time_ns is not None:
        _TRACE["exec_ns"].append(res.exec_time_ns)
    return res.results


def kernel(hidden_states, Wq, Wk, Wv, Wo, Wq_ind, Wk_ind, head_weights,
           temperature_param):
    hidden_states = np.asarray(hidden_states, dtype=FP32)
    Wq, Wk, Wv, Wo = (np.asarray(a, dtype=FP32) for a in (Wq, Wk, Wv, Wo))
    Wq_ind = np.asarray(Wq_ind, dtype=FP32)
    Wk_ind = np.asarray(Wk_ind, dtype=FP32)
    head_weights = np.asarray(head_weights, dtype=FP32)

    B, S, H = hidden_states.shape
    assert B == 1 and H == HIDDEN
    CS = H // N_CORES
    hidT = np.ascontiguousarray(hidden_states[0].T)
    # host-fused indexer weights (fp64 for exactness)
    Wfq = (Wq.astype(np.float64) @ Wq_ind.astype(np.float64)).astype(FP32)
    Wfk = (Wk.astype(np.float64) @ Wk_ind.astype(np.float64)).astype(FP32)

    # ---- launch A ----
    nca = _get("a", build_a, S, H, CS)
    ina = [
        {
            "hidT": hidT,
            "wq": np.ascontiguousarray(Wq[:, c * CS:(c + 1) * CS]),
            "wk": np.ascontiguousarray(Wk[:, c * CS:(c + 1) * CS]),
            "wv": np.ascontiguousarray(Wv[:, c * CS:(c + 1) * CS]),
            "wfq": np.ascontiguousarray(Wfq[:, c * CS:(c + 1) * CS]),
            "wfk": np.ascontiguousarray(Wfk[:, c * CS:(c + 1) * CS]),
        }
        for c in range(N_CORES)
    ]
    ra = _run(nca, ina)

    rel = np.zeros(S, dtype=np.float64)
    for c in range(N_CORES):
        rel += float(head_weights[c]) * ra[c]["rel"].astype(np.float64)
    # exp(-temp) scaling is monotone; irrelevant for top-k selection.

    k_sel = min(MAX_SELECTED, S)
    top_idx = np.argpartition(-rel, k_sel - 1)[:k_sel]
    selected = np.zeros(S, dtype=bool)
    selected[top_idx] = True

    # ---- launch B ----
    BIG = float(2 * S + 1024)
    hi = np.where(selected, BIG, np.arange(S, dtype=np.float64) + LOCAL_WINDOW)
    inb = [
        {
            "qTh": ra[c]["qT"],
            "kTh": ra[c]["kT"],
            "vTh": ra[c]["vT"],
            "woh": Wo[c * CS:(c + 1) * CS].astype(np.float16),
            "hivec": hi.astype(np.float16),
            "selv": selected.astype(np.float16),
        }
        for c in range(N_CORES)
    ]
    ncb = _get("b", build_b, S, H, NUM_HEADS // N_CORES, HEAD_DIM, LOCAL_WINDOW)
    rb = _run(ncb, inb)
    out = np.zeros((S, H), dtype=np.float32)
    for c in range(N_CORES):
        out += rb[c]["part"].astype(np.float32)
    return out.reshape(B, S, H)


# revision 32
# speedup vs baseline: 1.2015x; 1.0177x over previous
"""DeepSeek sparse attention on 8 Trainium2 NeuronCores (Bass/Tile).

Two SPMD launches:

  A (projections + lightning indexer, column/head-parallel): core c computes
     the 256-column slice (= its 2 attention heads) of q/k/v as fp16 (256,S)
     from f32r hidden^T resident in SBUF, PLUS the indexer-head-c projections
     qp_c/kp_c (256,S) using HOST-FUSED weights Wq@Wq_ind / Wk@Wk_ind (f32r,
     full precision — the top-k selection needs ~1e-5 relative accuracy), and
     rel_c[t] = sum_k relu(qp_c[t] . kp_c[k]) via PE + one relu-accumulate
     ACT pass per 128-token tile. Indexer passes run first so they overlap
     the hidden DMA window; scores interleave with the q/k/v passes.
  host: rel = sum_c w_c rel_c; top-1024 -> selected mask; hi/sel vectors.
  B (attention, head-parallel): core c feeds ITS OWN fp16 q/k/v slices from
     launch A straight back (no concat), computes causal/local/selected
     masked softmax attention for heads 2c,2c+1 and the partial output
     projection (S,H) in fp16; host sums the 8 fp16 partials in fp32.

All matmuls f32r or fp16 (1 PE cycle/row at N=512). fp16 everywhere in B
(calibrated: bf16 q/k/v + 16-bit partial store => 2.9e-3 rel err; fp16 is
strictly tighter; indexer stays f32r — one top-k swap costs ~1.5e-2).
"""

import math

import numpy as np

import concourse.bass as bass
from concourse import bass_isa
import concourse.mybir as mybir
from concourse import bacc
from concourse.tile import TileContext
from concourse.masks import make_identity
from concourse.bass_utils import run_bass_kernel_spmd

# Problem constants (hardcoded per contract)
HIDDEN = 2048
NUM_HEADS = 16
HEAD_DIM = 128
NUM_IND_HEADS = 8
IND_DIM = HIDDEN // NUM_IND_HEADS  # 256
MAX_SELECTED = 1024
LOCAL_WINDOW = 512
N_CORES = 8

F32 = mybir.dt.float32
F32R = mybir.dt.float32r
F16 = mybir.dt.float16
FP32 = np.float32

_TRACE = {"on": False, "exec_ns": []}


def build_a(S=2048, H=HIDDEN, CS=HIDDEN // N_CORES):
    """Per-core: q/k/v column slices (CS,S) fp16 + indexer rel_c (S) f32."""
    nc = bacc.Bacc("TRN2", target_bir_lowering=False, debug=False)
    HT, MC, NQ, QT = H // 128, CS // 128, S // 512, S // 128
    hidT = nc.dram_tensor("hidT", [H, S], F32R, kind="ExternalInput")
    wq = nc.dram_tensor("wq", [H, CS], F32R, kind="ExternalInput")
    wk = nc.dram_tensor("wk", [H, CS], F32R, kind="ExternalInput")
    wv = nc.dram_tensor("wv", [H, CS], F32R, kind="ExternalInput")
    wfq = nc.dram_tensor("wfq", [H, CS], F32R, kind="ExternalInput")
    wfk = nc.dram_tensor("wfk", [H, CS], F32R, kind="ExternalInput")
    qT = nc.dram_tensor("qT", [CS, S], F16, kind="ExternalOutput")
    kT = nc.dram_tensor("kT", [CS, S], F16, kind="ExternalOutput")
    vT = nc.dram_tensor("vT", [CS, S], F16, kind="ExternalOutput")
    rel = nc.dram_tensor("rel", [S], F32, kind="ExternalOutput")

    G = 8          # hidden chunks
    TG = HT // G   # strips per chunk

    with TileContext(nc) as tc:
        with (
            tc.tile_pool(name="hid", bufs=1) as hpool,
            tc.tile_pool(name="wt", bufs=2) as wpool,
            tc.tile_pool(name="proj", bufs=1) as ppool,
            tc.tile_pool(name="st", bufs=2) as stpool,
            tc.tile_pool(name="scr", bufs=1) as scrpool,
            tc.tile_pool(name="rm", bufs=1) as rmpool,
            tc.tile_pool(name="ps", bufs=1, space="PSUM") as pspool,
        ):
            # ---- input DMAs. Order matters: the first matmul needs the
            # first half of wfq plus hidden chunk 0, so those go first; wfk
            # comes after the hidden chunks (first needed ~50us in); wq/wk/wv
            # are issued later, at the program points where their weight-pool
            # slot is freed (avoids WAR stalls on the slot).
            def load_w(wdram, dt=F32R, halves=1):
                wr = wpool.tile([128, HT * CS], dt, tag="w", name="w")
                hh = HT // halves
                for i in range(halves):
                    nc.sync.dma_start(
                        out=wr[:, i * hh * CS:(i + 1) * hh * CS].rearrange(
                            "p (t c) -> p t c", t=hh
                        ),
                        in_=wdram[i * hh * 128:(i + 1) * hh * 128, :].rearrange(
                            "(t p) c -> p t c", p=128
                        ),
                    )
                return wr

            # hidden chunks: strips [1,1,2,2,2,2,2,2,2] so the first
            # matmul only waits on one 1MB strip; wfq loads in quarters.
            CHUNKS = [1, 1] + [2] * 7
            CUM = [0]
            for n in CHUNKS:
                CUM.append(CUM[-1] + n)
            strip2chunk = {}
            for g, n in enumerate(CHUNKS):
                for t in range(CUM[g], CUM[g + 1]):
                    strip2chunk[t] = (g, t - CUM[g])
            hidc = [hpool.tile([128, CHUNKS[g] * S], F32R, name=f"hidc{g}")
                    for g in range(len(CHUNKS))]

            def load_hid(g):
                n = CHUNKS[g]
                nc.sync.dma_start(
                    out=hidc[g].rearrange("p (t s) -> p t s", t=n),
                    in_=hidT[CUM[g] * 128:CUM[g + 1] * 128, :].rearrange(
                        "(t p) s -> p t s", p=128
                    ),
                )

            wfq_t = wpool.tile([128, HT * CS], F32R, tag="w", name="w")
            QQ = HT // 4
            nc.sync.dma_start(
                out=wfq_t[:, :QQ * CS].rearrange("p (t c) -> p t c", t=QQ),
                in_=wfq[:QQ * 128, :].rearrange("(t p) c -> p t c", p=128),
            )
            load_hid(0)
            nc.sync.dma_start(
                out=wfq_t[:, QQ * CS:2 * QQ * CS].rearrange(
                    "p (t c) -> p t c", t=QQ),
                in_=wfq[QQ * 128:2 * QQ * 128, :].rearrange(
                    "(t p) c -> p t c", p=128),
            )
            load_hid(1)
            nc.sync.dma_start(
                out=wfq_t[:, 2 * QQ * CS:].rearrange(
                    "p (t c) -> p t c", t=2 * QQ),
                in_=wfq[2 * QQ * 128:, :].rearrange(
                    "(t p) c -> p t c", p=128),
            )
            for g in range(2, len(CHUNKS)):
                load_hid(g)
            wfk_t = load_w(wfk)

            # resident f32r indexer projections qp^T/kp^T (2 x 128 x S each)
            qpt = [ppool.tile([128, S], F32R, name=f"qpt{m}") for m in range(MC)]
            kpt = [ppool.tile([128, S], F32R, name=f"kpt{m}") for m in range(MC)]

            # psum regions: 4 banks for projection passes ("pj"), and one
            # [128, S] region ("scr") that serves double duty: the kp passes
            # accumulate in its 512-slices, and the indexer-score tiles use
            # it whole.
            def pj_psums():
                return [
                    pspool.tile([128, 512], F32, tag=f"pj{i}", name=f"pj{i}")
                    for i in range(NQ)
                ]

            def scr_psum():
                return pspool.tile([128, S], F32, tag="scr", name="scr")

            relmat = rmpool.tile([128, QT], F32, name="relmat")
            scratch = scrpool.tile([128, S], F16, name="scratch")

            score_state = {"next": 0}

            def emit_score_qt():
                """Indexer scores for one 128-token tile: 8 matmuls into the
                scr psum region + one relu-accumulate -> relmat column."""
                qt = score_state["next"]
                if qt >= QT:
                    return False
                score_state["next"] += 1
                sps = scr_psum()
                for d in range(MC):
                    for kc in range(NQ):
                        nc.tensor.matmul(
                            sps[:, kc * 512:(kc + 1) * 512],
                            qpt[d][:, qt * 128:(qt + 1) * 128],
                            kpt[d][:, kc * 512:(kc + 1) * 512],
                            start=(d == 0), stop=(d == MC - 1),
                        )
                nc.scalar.activation(
                    scratch, sps, mybir.ActivationFunctionType.Relu,
                    accum_out=relmat[:, qt:qt + 1],
                )
                return True

            def sl(psums, qc):
                return (psums[qc] if isinstance(psums, list)
                        else psums[:, qc * 512:(qc + 1) * 512])

            def proj_passes(groups, score_slots=()):
                """Interleaved m-tile passes: each group = (wtile, mc, psums,
                finish). Strips advance together so every group progresses
                chunk-by-chunk behind the hidden DMA."""
                for t in range(HT):
                    for wtile, mc, psums, _ in groups:
                        lhsT = wtile[:, t * CS + mc * 128:
                                     t * CS + mc * 128 + 128]
                        g, tl = strip2chunk[t]
                        rhs = hidc[g]
                        for qc in range(NQ):
                            nc.tensor.matmul(
                                sl(psums, qc), lhsT,
                                rhs[:, tl * S + qc * 512:
                                    tl * S + qc * 512 + 512],
                                start=(t == 0), stop=(t == HT - 1),
                            )
                    if t in score_slots:
                        emit_score_qt()
                for _, _, _, finish in groups:
                    finish()

            def copy_to(dst, psums):
                def fin():
                    for qc in range(NQ):
                        eng = nc.vector if qc % 2 == 0 else nc.scalar
                        if eng is nc.vector:
                            nc.vector.tensor_copy(
                                dst[:, qc * 512:(qc + 1) * 512], sl(psums, qc)
                            )
                        else:
                            nc.scalar.copy(
                                dst[:, qc * 512:(qc + 1) * 512], sl(psums, qc)
                            )
                return fin

            # ---- indexer projection passes; the qp-m0/m1 pair overlaps the
            # hidden-DMA window (both read wfq, which arrives first).
            pj = pj_psums()
            sc = scr_psum()
            proj_passes([
                (wfq_t, 0, pj, copy_to(qpt[0], pj)),
                (wfq_t, 1, sc, copy_to(qpt[1], sc)),
            ])
            # wfq slot free now -> issue wq load
            wq_t = load_w(wq)
            pj = pj_psums()
            sc = scr_psum()
            proj_passes([
                (wfk_t, 0, pj, copy_to(kpt[0], pj)),
                (wfk_t, 1, sc, copy_to(kpt[1], sc)),
            ])
            wk_t = load_w(wk)

            # ---- q/k/v passes (fp16 weights) with indexer scores interleaved
            def store_pass(wtile, mc, odram):
                psums = pj_psums()
                stage = stpool.tile([128, S], F16, tag="st", name="st")

                def fin():
                    for qc in range(NQ):
                        if qc % 2 == 0:
                            nc.vector.tensor_copy(
                                stage[:, qc * 512:(qc + 1) * 512], psums[qc]
                            )
                        else:
                            nc.scalar.copy(
                                stage[:, qc * 512:(qc + 1) * 512], psums[qc]
                            )
                    nc.sync.dma_start(
                        out=odram[mc * 128:(mc + 1) * 128, :], in_=stage
                    )
                proj_passes([(wtile, mc, psums, fin)],
                            score_slots=(1, 6, 11))

            store_pass(wq_t, 0, qT)
            store_pass(wq_t, 1, qT)
            wv_t = load_w(wv)
            store_pass(wk_t, 0, kT)
            store_pass(wk_t, 1, kT)
            store_pass(wv_t, 0, vT)
            store_pass(wv_t, 1, vT)
            while emit_score_qt():
                pass

            nc.sync.dma_start(
                out=rel.rearrange("(t p) -> p t", p=128), in_=relmat
            )
    nc.compile()
    return nc


def build_b(S=2048, H=HIDDEN, NHC=NUM_HEADS // N_CORES, HD=HEAD_DIM,
            window=LOCAL_WINDOW):
    """Per-core attention for 2 heads + fp16 partial output projection."""
    nc = bacc.Bacc("TRN2", target_bir_lowering=False, debug=False)
    KC, NQ, QT, OCC = S // 128, S // 512, S // 128, H // 512
    qTh = nc.dram_tensor("qTh", [NHC * HD, S], F16, kind="ExternalInput")
    kTh = nc.dram_tensor("kTh", [NHC * HD, S], F16, kind="ExternalInput")
    vTh = nc.dram_tensor("vTh", [NHC * HD, S], F16, kind="ExternalInput")
    woh = nc.dram_tensor("woh", [NHC * HD, H], F16, kind="ExternalInput")
    hivec = nc.dram_tensor("hivec", [S], F16, kind="ExternalInput")
    selv = nc.dram_tensor("selv", [S], F16, kind="ExternalInput")
    part = nc.dram_tensor("part", [S, H], F16, kind="ExternalOutput")

    scale = 1.0 / math.sqrt(HD)
    AF = mybir.ActivationFunctionType
    OP = mybir.AluOpType

    with TileContext(nc) as tc:
        with (
            tc.tile_pool(name="const", bufs=1) as cpool,
            tc.tile_pool(name="qk", bufs=1) as qkpool,
            tc.tile_pool(name="vt", bufs=2) as vtpool,
            tc.tile_pool(name="vh", bufs=1) as vhpool,
            tc.tile_pool(name="et", bufs=6) as etpool,
            tc.tile_pool(name="aon", bufs=1) as aopool,
            tc.tile_pool(name="dr", bufs=2) as drpool,
            tc.tile_pool(name="ost", bufs=2) as ostpool,
            tc.tile_pool(name="ps", bufs=1, space="PSUM") as pspool,
        ):
            # v first (transposes are at the head of the PE queue), then q/k
            # of head 0 so scoring starts right behind the transposes.
            vts0 = vtpool.tile([128, S], F16, tag="vts", name="vts")
            nc.sync.dma_start(out=vts0, in_=vTh[0:HD, :])
            qsb, ksb = [], []
            for h in range(NHC):
                k = qkpool.tile([128, S], F16, name=f"ksb{h}")
                nc.sync.dma_start(out=k, in_=kTh[h * HD:(h + 1) * HD, :])
                ksb.append(k)
                q = qkpool.tile([128, S], F16, name=f"qsb{h}")
                nc.sync.dma_start(out=q, in_=qTh[h * HD:(h + 1) * HD, :])
                qsb.append(q)

            hvec = cpool.tile([128, KC], F16, name="hvec")
            nc.sync.dma_start(out=hvec, in_=hivec.rearrange("(t p) -> p t", p=128))
            svec = cpool.tile([128, KC], F16, name="svec")
            nc.sync.dma_start(out=svec, in_=selv.rearrange("(t p) -> p t", p=128))

            wsb = []
            for h in range(NHC):
                w = qkpool.tile([128, H], F16, name=f"wsb{h}")
                nc.sync.dma_start(out=w, in_=woh[h * HD:(h + 1) * HD, :])
                wsb.append(w)

            svec32 = cpool.tile([128, KC], F32, name="svec32")
            nc.vector.tensor_copy(svec32, svec)
            ones = cpool.tile([128, 1], F16, name="ones")
            nc.vector.memset(ones, 1.0)
            ident = cpool.tile([128, 128], F16, name="ident")
            make_identity(nc, ident)
            iota = cpool.tile([128, S], F16, name="iota")
            nc.gpsimd.iota(
                iota, pattern=[[1, S]], base=0, channel_multiplier=0,
                allow_small_or_imprecise_dtypes=True,
            )
            kvec = cpool.tile([128, KC], F16, name="kvec")
            nc.gpsimd.iota(
                kvec, pattern=[[128, KC]], base=0, channel_multiplier=1,
                allow_small_or_imprecise_dtypes=True,
            )

            aon = [aopool.tile([128, S], F16, name=f"aon{h}") for h in range(NHC)]
            vhf = [vhpool.tile([128, S], F16, name=f"vhf{h}") for h in range(NHC)]
            vsl = [vhpool.tile([128, S], F16, name=f"vsl{h}") for h in range(NHC)]

            def normalize(h, qc, avp, den):
                q0 = qc * 512
                dq = drpool.tile([1, 512], F32, tag="dq", name="dq")
                nc.scalar.copy(dq, den[0:1, :])
                rq = drpool.tile([1, 512], F32, tag="rq", name="rq")
                nc.vector.reciprocal_approx_fast(rq, dq)
                rbs = drpool.tile([128, 512], F32, tag="rbs", name="rbs")
                nc.gpsimd.partition_broadcast(rbs, rq)
                nc.vector.scalar_tensor_tensor(
                    aon[h][:, q0:q0 + 512], rbs, 1.0, avp,
                    op0=OP.mult, op1=OP.mult,
                )

            def outproj(qc):
                """Output projection for the 4 query tiles of chunk qc,
                accumulating both heads; fp16 stage -> one DMA per tile."""
                for qt in range(qc * NQ, qc * NQ + NQ):
                    ostage = ostpool.tile([128, H], F16, tag="ost", name="ost")
                    for oc in range(OCC):
                        wop = pspool.tile([128, 512], F32, tag="wo", bufs=2,
                                          name="wo")
                        for h in range(NHC):
                            nc.tensor.matmul(
                                wop, aon[h][:, qt * 128:(qt + 1) * 128],
                                wsb[h][:, oc * 512:(oc + 1) * 512],
                                start=(h == 0), stop=(h == NHC - 1),
                            )
                        if oc % 2 == 0:
                            nc.vector.tensor_copy(
                                ostage[:, oc * 512:(oc + 1) * 512], wop
                            )
                        else:
                            nc.scalar.copy(
                                ostage[:, oc * 512:(oc + 1) * 512], wop
                            )
                    nc.sync.dma_start(
                        out=part[qt * 128:(qt + 1) * 128, :], in_=ostage
                    )

            for h in range(NHC):
                if h == 0:
                    vts = vts0
                else:
                    vts = vtpool.tile([128, S], F16, tag="vts", name="vts")
                    nc.sync.dma_start(out=vts, in_=vTh[h * HD:(h + 1) * HD, :])

                def transpose_batch(kcs):
                    # v tiles -> (k, hd) layout; vsl = v * selected. The tp
                    # psum shares the "wo" tag (outproj is temporally
                    # disjoint); copies go to gpsimd, which is mostly idle.
                    for kc in kcs:
                        tp = pspool.tile([128, 128], F16, tag="wo", bufs=2,
                                         name="tp")
                        nc.tensor.transpose(
                            tp, vts[:, kc * 128:(kc + 1) * 128], ident
                        )
                        if kc % 2 == 0:
                            nc.vector.tensor_copy(
                                vhf[h][:, kc * 128:(kc + 1) * 128], tp
                            )
                        else:
                            nc.scalar.copy(
                                vhf[h][:, kc * 128:(kc + 1) * 128], tp
                            )
                        nc.vector.tensor_scalar_mul(
                            vsl[h][:, kc * 128:(kc + 1) * 128],
                            vhf[h][:, kc * 128:(kc + 1) * 128],
                            svec32[:, kc:kc + 1],
                        )

                for qc in range(NQ):
                    q0 = qc * 512
                    kcm = (q0 + 511) // 128  # last causal k-tile
                    avp = pspool.tile([128, 512], F32, tag="av", bufs=2,
                                      name="avp")
                    den = pspool.tile([128, 512], F32, tag="den", bufs=1,
                                      name="den")

                    def score_tile(kc):
                        k0 = kc * 128
                        far = q0 > k0 + 127 + window
                        sps = pspool.tile([128, 512], F32, tag="sc", bufs=3,
                                          name="sps")
                        nc.tensor.matmul(
                            sps, ksb[h][:, k0:k0 + 128],
                            qsb[h][:, q0:q0 + 512],
                            start=True, stop=True,
                        )
                        et = etpool.tile([128, 512], F16, tag="et", name="et")
                        nc.scalar.activation(et, sps, AF.Exp, scale=scale)
                        if q0 < k0 + 128:
                            nc.vector.scalar_tensor_tensor(
                                et, iota[:, q0:q0 + 512], kvec[:, kc:kc + 1],
                                et, op0=OP.is_ge, op1=OP.mult,
                            )
                        elif not far and q0 + 511 > k0 + window:
                            nc.vector.scalar_tensor_tensor(
                                et, iota[:, q0:q0 + 512], hvec[:, kc:kc + 1],
                                et, op0=OP.is_le, op1=OP.mult,
                            )
                        return et, far

                    def av_den(kc, et, far):
                        k0 = kc * 128
                        nc.tensor.matmul(
                            avp, (vsl if far else vhf)[h][:, k0:k0 + 128], et,
                            start=(kc == 0), stop=(kc == kcm),
                        )
                        nc.tensor.matmul(
                            den[0:1, :],
                            svec[:, kc:kc + 1] if far else ones, et,
                            start=(kc == 0), stop=(kc == kcm),
                            tile_position=(0, 0),
                        )

                    if qc == 0:
                        # scores first: they only need q/k, so the PE can
                        # work while v arrives and the transposes run.
                        ets = [score_tile(kc) for kc in range(kcm + 1)]
                        transpose_batch(range(qc * NQ, qc * NQ + NQ))
                        for kc, (et, far) in enumerate(ets):
                            av_den(kc, et, far)
                    else:
                        transpose_batch(range(qc * NQ, qc * NQ + NQ))
                        for kc in range(kcm + 1):
                            et, far = score_tile(kc)
                            av_den(kc, et, far)
                    normalize(h, qc, avp, den)
                    if h == NHC - 1:
                        outproj(qc)
    nc.compile()
    return nc


_CACHE = {}


def _get(name, builder, *args):
    key = (name,) + args
    if key not in _CACHE:
        _CACHE[key] = builder(*args)
    return _CACHE[key]


def _run(nc, in_maps):
    res = run_bass_kernel_spmd(
        nc, in_maps, core_ids=list(range(N_CORES)), trace=_TRACE["on"]
    )
    if _TRACE["on"] and res.exec_time_ns is not None:
        _TRACE["exec_ns"].append(res.exec_time_ns)
    return res.results


def kernel(hidden_states, Wq, Wk, Wv, Wo, Wq_ind, Wk_ind, head_weights,
           temperature_param):
    hidden_states = np.asarray(hidden_states, dtype=FP32)
    Wq, Wk, Wv, Wo = (np.asarray(a, dtype=FP32) for a in (Wq, Wk, Wv, Wo))
    Wq_ind = np.asarray(Wq_ind, dtype=FP32)
    Wk_ind = np.asarray(Wk_ind, dtype=FP32)
    head_weights = np.asarray(head_weights, dtype=FP32)

    B, S, H = hidden_states.shape
    assert B == 1 and H == HIDDEN
    CS = H // N_CORES
    hidT = np.ascontiguousarray(hidden_states[0].T)
    # host-fused indexer weights (fp64 for exactness)
    Wfq = (Wq.astype(np.float64) @ Wq_ind.astype(np.float64)).astype(FP32)
    Wfk = (Wk.astype(np.float64) @ Wk_ind.astype(np.float64)).astype(FP32)

    # ---- launch A ----
    nca = _get("a", build_a, S, H, CS)
    ina = [
        {
            "hidT": hidT,
            "wq": np.ascontiguousarray(Wq[:, c * CS:(c + 1) * CS]),
            "wk": np.ascontiguousarray(Wk[:, c * CS:(c + 1) * CS]),
            "wv": np.ascontiguousarray(Wv[:, c * CS:(c + 1) * CS]),
            "wfq": np.ascontiguousarray(Wfq[:, c * CS:(c + 1) * CS]),
            "wfk": np.ascontiguousarray(Wfk[:, c * CS:(c + 1) * CS]),
        }
        for c in range(N_CORES)
    ]
    ra = _run(nca, ina)

    rel = np.zeros(S, dtype=np.float64)
    for c in range(N_CORES):
        rel += float(head_weights[c]) * ra[c]["rel"].astype(np.float64)
    # exp(-temp) scaling is monotone; irrelevant for top-k selection.

    k_sel = min(MAX_SELECTED, S)
    top_idx = np.argpartition(-rel, k_sel - 1)[:k_sel]
    selected = np.zeros(S, dtype=bool)
    selected[top_idx] = True

    # ---- launch B ----
    BIG = float(2 * S + 1024)
    hi = np.where(selected, BIG, np.arange(S, dtype=np.float64) + LOCAL_WINDOW)
    inb = [
        {
            "qTh": ra[c]["qT"],
            "kTh": ra[c]["kT"],
            "vTh": ra[c]["vT"],
            "woh": Wo[c * CS:(c + 1) * CS].astype(np.float16),
            "hivec": hi.astype(np.float16),
            "selv": selected.astype(np.float16),
        }
        for c in range(N_CORES)
    ]
    ncb = _get("b", build_b, S, H, NUM_HEADS // N_CORES, HEAD_DIM, LOCAL_WINDOW)
    rb = _run(ncb, inb)
    out = np.zeros((S, H), dtype=np.float32)
    for c in range(N_CORES):
        out += rb[c]["part"].astype(np.float32)
    return out.reshape(B, S, H)
